# revision 1
# baseline (speedup 1.0000x reference)
"""CTC loss kernel for Trainium2 (8 NeuronCores, batch-parallel).

Strategy
--------
Batch B=64 is sharded 8 samples/core. Per core, the memory-bound part streams
pred [8,160,6625] f32 once through SBUF in ten [128, 6625] tiles laid out
time-major (partition p = b*16 + t_inner):

  1. DMA tile in (HWDGE, ~3.4 MB)
  2. ScalarE: in-place Exp with fused per-row accumulate -> softmax denominator s
  3. VectorE: r = 1/s
  4. GPSIMD ap_gather: pick the 51 extended-label columns per row (indices are
     per-sample, shared across each 16-partition group)
  5. VectorE scalar_tensor_tensor: p = gathered * r * maskK, where maskK bakes
     in the K=C scale factor and zeroes states beyond each sample's final CTC
     state (2*target_len) - those can never influence the result (transitions
     only move forward in s) and masking them keeps the linear-domain DP in
     f32 range (validated: final-state/max ratio stays >= ~0.4).
  6. SBUF->SBUF DMA regroups partitions (b*16+t) -> per-time-step [8, 51] rows.

The CTC forward recursion then runs on VectorE in the *linear* domain
(probabilities scaled by K, renormalized by the running sum every 8 steps; the
normalizers c_j are written out and folded back on the host in f64):

  alpha_new[s] = (alpha[s] + alpha[s-1] + skip[s]*alpha[s-2]) * p[t, s]

implemented with two guard columns so the shifts are plain free-dim slices.
Outputs per core: final alpha [8, 64] and normalizers [8, 20]. The host
computes -log(alpha[2L] + alpha[2L-1]) + corrections, zero-infinity, the
length division and the batch mean (a 64-element epilogue, f64).
"""

import math
from contextlib import ExitStack

import numpy as np

import concourse.bass as bass
import concourse.tile as tile
from concourse import bacc, mybir
from concourse.bass_utils import run_bass_kernel_spmd

N_CORES = 8
B = 64
T = 160
C = 6625
L = 25
S = 2 * L + 1           # 51 extended states
BPC = B // N_CORES      # 8 samples per core
TBLK = 16               # time steps per streamed tile
NBLK = T // TBLK        # 10 tiles per core
GC = 64                 # gather columns (51 states padded to 64)
NORM_EVERY = 8
NNORM = len([t for t in range(1, T) if t % NORM_EVERY == NORM_EVERY - 1])  # 20
K_SCALE = float(C)

FP = mybir.dt.float32
MULT = mybir.AluOpType.mult
ADD = mybir.AluOpType.add


def build_nc() -> bass.Bass:
    nc = bacc.Bacc("TRN2", target_bir_lowering=False, debug=False,
                   num_devices=N_CORES)
    pred = nc.dram_tensor("pred", [BPC, T, C], FP, kind="ExternalInput")
    idx = nc.dram_tensor("idx", [128, GC // 16], mybir.dt.int16, kind="ExternalInput")
    maskv = nc.dram_tensor("maskv", [BPC, GC], FP, kind="ExternalInput")
    maskk = nc.dram_tensor("maskk", [128, GC], FP, kind="ExternalInput")
    out_alpha = nc.dram_tensor("out_alpha", [BPC, GC], FP, kind="ExternalOutput")
    out_c = nc.dram_tensor("out_c", [BPC, NNORM], FP, kind="ExternalOutput")

    with tile.TileContext(nc) as tc, ExitStack() as ctx:
        pred_pool = ctx.enter_context(tc.tile_pool(name="pred_pool", bufs=3))
        small = ctx.enter_context(tc.tile_pool(name="small", bufs=3))

        def single(shape, dtype, name):
            t, free = tc.tile(shape, dtype, name=name)
            ctx.callback(free)
            return t

        idx_sb = single([128, GC // 16], mybir.dt.int16, "idx_sb")
        maskv_sb = single([BPC, GC], FP, "maskv_sb")
        maskk_sb = single([128, GC], FP, "maskk_sb")
        # ping/pong alpha with 2 guard columns each: ping states at 2..52,
        # pong states at 66..116; guards stay zero forever.
        alpha = single([BPC, 128], FP, "alpha")
        cbuf = single([BPC, NNORM], FP, "cbuf")
        rcn = single([BPC, 1], FP, "rcn")
        pdp = [single([BPC, TBLK, GC], FP, f"pdp{k}") for k in range(NBLK)]
        dram_pool = ctx.enter_context(
            tc.tile_pool(name="pscr_pool", bufs=1, space="DRAM"))
        pscr = [dram_pool.tile([BPC, TBLK, GC], FP, name=f"pscr{k}")
                for k in range(NBLK)]

        nc.sync.dma_start(out=idx_sb[:, :], in_=idx[:, :])
        nc.sync.dma_start(out=maskv_sb[:, :], in_=maskv[:, :])
        nc.sync.dma_start(out=maskk_sb[:, :], in_=maskk[:, :])
        nc.vector.memset(alpha[:, :], 0.0)

        PING, PONG = 0, 64
        jn = 0
        for k in range(NBLK):
            pt = pred_pool.tile([128, C], FP, tag="pt")
            # plain 2D out AP: flat element order of in_ is (b, t, c) row-major,
            # which lands as partition p = b*16 + t  (b-major within the tile)
            nc.sync.dma_start(
                out=pt[:, :],
                in_=pred[:, k * TBLK:(k + 1) * TBLK, :],
            )
            s_k = small.tile([128, 1], FP, tag="s_k")
            nc.scalar.activation(
                out=pt[:, :], in_=pt[:, :],
                func=mybir.ActivationFunctionType.Exp,
                accum_out=s_k[:, :],
            )
            r_k = small.tile([128, 1], FP, tag="r_k")
            nc.vector.reciprocal(r_k[:, :], s_k[:, :])
            g_k = small.tile([128, GC], FP, tag="g_k")
            nc.gpsimd.ap_gather(
                g_k[:, :], pt[:, :], idx_sb[:, :],
                channels=128, num_elems=C, d=1, num_idxs=GC,
            )
            pg_k = small.tile([128, GC], FP, tag="pg_k")
            # (scalar_tensor_tensor / tensor_tensor_reduce crash the DVE exec
            # unit on this runtime - use standard two-op forms instead)
            nc.vector.tensor_scalar_mul(pg_k[:, :], g_k[:, :], r_k[:, 0:1])
            nc.vector.tensor_mul(pg_k[:, :], pg_k[:, :], maskk_sb[:, :])
            # partition regroup (b*16+t, s) -> (b, t, s) via DRAM scratch:
            # both DMAs use plain APs (partition-split SBUF APs miscompile)
            nc.scalar.dma_start(out=pscr[k][:, :, :], in_=pg_k[:, :])
            nc.scalar.dma_start(out=pdp[k][:, :, :], in_=pscr[k][:, :, :])

            for ti in range(TBLK):
                t = k * TBLK + ti
                if t == 0:
                    # alpha0: states 0,1 get p[0, 0:2]
                    nc.vector.tensor_copy(
                        alpha[:, PING + 2:PING + 4], pdp[0][:, 0, 0:2]
                    )
                    continue
                src = PING if t % 2 == 1 else PONG
                dst = PONG if t % 2 == 1 else PING
                vt = small.tile([BPC, S], FP, tag="vt")
                nc.vector.tensor_mul(
                    vt[:, :], alpha[:, src:src + S], maskv_sb[:, 0:S]
                )
                ut = small.tile([BPC, S], FP, tag="ut")
                nc.vector.tensor_add(
                    ut[:, :], alpha[:, src + 2:src + 2 + S],
                    alpha[:, src + 1:src + 1 + S],
                )
                nc.vector.tensor_add(ut[:, :], ut[:, :], vt[:, :])
                pcur = pdp[k][:, ti, 0:S]
                adst = alpha[:, dst + 2:dst + 2 + S]
                if t % NORM_EVERY == NORM_EVERY - 1:
                    nc.vector.tensor_mul(adst, ut[:, :], pcur)
                    nc.vector.tensor_reduce(
                        out=cbuf[:, jn:jn + 1], in_=adst,
                        axis=mybir.AxisListType.X, op=ADD,
                    )
                    nc.vector.reciprocal(rcn[:, :], cbuf[:, jn:jn + 1])
                    nc.vector.tensor_scalar_mul(adst, adst, rcn[:, 0:1])
                    jn += 1
                else:
                    nc.vector.tensor_mul(adst, ut[:, :], pcur)

        assert jn == NNORM
        # final alpha lives in PONG half (t=159 is odd)
        nc.sync.dma_start(out=out_alpha[:, :], in_=alpha[:, PONG:PONG + GC])
        nc.sync.dma_start(out=out_c[:, :], in_=cbuf[:, :])
    nc.compile()
    return nc


_CACHE: dict = {}


def _get_nc() -> bass.Bass:
    if "nc" not in _CACHE:
        _CACHE["nc"] = build_nc()
    return _CACHE["nc"]


LAST_RESULTS = None


def kernel(pred, targets, targets_lengths) -> np.ndarray:
    global LAST_RESULTS
    pred = np.ascontiguousarray(np.asarray(pred, dtype=np.float32))
    targets = np.asarray(targets).astype(np.int64)
    tl = np.asarray(targets_lengths).astype(np.int64)
    assert pred.shape == (B, T, C), pred.shape
    assert targets.shape == (B, L)

    # host prep: extended labels, skip mask, gather indices, band/scale mask
    ext = np.zeros((B, S), dtype=np.int64)
    ext[:, 1::2] = targets
    skip = np.zeros((B, S), dtype=np.float32)
    skip[:, 2:] = ((ext[:, 2:] != 0) & (ext[:, 2:] != ext[:, :-2])).astype(np.float32)

    in_maps = []
    for c in range(N_CORES):
        lo = c * BPC
        idx16 = np.zeros((128, GC // 16), dtype=np.int16)
        maskv = np.zeros((BPC, GC), dtype=np.float32)
        maskk = np.zeros((128, GC), dtype=np.float32)
        for g in range(BPC):
            b = lo + g
            for j in range(S):
                idx16[16 * g + (j % 16), j // 16] = ext[b, j]
            maskv[g, :S] = skip[b]
            band_hi = 2 * int(tl[b])  # states 0..2*tl allowed
            maskk[16 * g:16 * (g + 1), :min(band_hi + 1, S)] = K_SCALE
        in_maps.append({
            "pred": np.ascontiguousarray(pred[lo:lo + BPC]),
            "idx": idx16,
            "maskv": maskv,
            "maskk": maskk,
        })

    nc = _get_nc()
    LAST_RESULTS = run_bass_kernel_spmd(nc, in_maps, core_ids=list(range(N_CORES)))
    results = LAST_RESULTS.results

    # host epilogue (f64, 64 elements)
    logK = math.log(K_SCALE)
    per_sample = np.zeros(B, dtype=np.float64)
    for c in range(N_CORES):
        a = results[c]["out_alpha"].astype(np.float64)   # [8, 64]
        cs = results[c]["out_c"].astype(np.float64)      # [8, 20]
        for g in range(BPC):
            b = c * BPC + g
            a1 = a[g, 2 + 2 * tl[b]]
            a2 = a[g, 2 + 2 * tl[b] - 1]
            tot = a1 + a2
            if tot <= 0.0 or np.any(cs[g] <= 0.0):
                raw = np.inf
            else:
                raw = -(np.log(tot) + np.sum(np.log(cs[g])) - T * logK)
            safe = 0.0 if np.isinf(raw) else raw
            per_sample[b] = safe / max(int(tl[b]), 1)
    return np.asarray(per_sample.mean(), dtype=np.float32)



# revision 4
# speedup vs baseline: 1.0508x; 1.0508x over previous
"""CTC loss kernel for Trainium2 (8 NeuronCores, batch-parallel).

Strategy
--------
Batch B=64 is sharded 8 samples/core. Per core, the memory-bound part streams
pred [8,160,6625] f32 once through SBUF in ten [128, 6625] tiles laid out
time-major (partition p = b*16 + t_inner):

  1. DMA tile in (HWDGE, ~3.4 MB)
  2. ScalarE: in-place Exp with fused per-row accumulate -> softmax denominator s
  3. VectorE: r = 1/s
  4. GPSIMD ap_gather: pick the 51 extended-label columns per row (indices are
     per-sample, shared across each 16-partition group)
  5. VectorE scalar_tensor_tensor: p = gathered * r * maskK, where maskK bakes
     in the K=C scale factor and zeroes states beyond each sample's final CTC
     state (2*target_len) - those can never influence the result (transitions
     only move forward in s) and masking them keeps the linear-domain DP in
     f32 range (validated: final-state/max ratio stays >= ~0.4).
  6. SBUF->SBUF DMA regroups partitions (b*16+t) -> per-time-step [8, 51] rows.

The CTC forward recursion then runs on VectorE in the *linear* domain
(probabilities scaled by K, renormalized by the running sum every 8 steps; the
normalizers c_j are written out and folded back on the host in f64):

  alpha_new[s] = (alpha[s] + alpha[s-1] + skip[s]*alpha[s-2]) * p[t, s]

implemented with two guard columns so the shifts are plain free-dim slices.
Outputs per core: final alpha [8, 64] and normalizers [8, 20]. The host
computes -log(alpha[2L] + alpha[2L-1]) + corrections, zero-infinity, the
length division and the batch mean (a 64-element epilogue, f64).
"""

import math
from contextlib import ExitStack

import numpy as np

import concourse.bass as bass
import concourse.tile as tile
from concourse import bacc, mybir
from concourse.bass_utils import run_bass_kernel_spmd

N_CORES = 8
B = 64
T = 160
C = 6625
L = 25
S = 2 * L + 1           # 51 extended states
BPC = B // N_CORES      # 8 samples per core
TBLK = 16               # time steps per streamed tile
NBLK = T // TBLK        # 10 tiles per core
GC = 64                 # gather columns (51 states padded to 64)
NORM_EVERY = 8
NNORM = len([t for t in range(1, T) if t % NORM_EVERY == NORM_EVERY - 1])  # 20
K_SCALE = float(C)

FP = mybir.dt.float32
MULT = mybir.AluOpType.mult
ADD = mybir.AluOpType.add


def build_nc() -> bass.Bass:
    nc = bacc.Bacc("TRN2", target_bir_lowering=False, debug=False,
                   num_devices=N_CORES)
    pred = nc.dram_tensor("pred", [BPC, T, C], FP, kind="ExternalInput")
    idx = nc.dram_tensor("idx", [128, GC // 16], mybir.dt.int16, kind="ExternalInput")
    maskv = nc.dram_tensor("maskv", [BPC, GC], FP, kind="ExternalInput")
    maskk = nc.dram_tensor("maskk", [128, GC], FP, kind="ExternalInput")
    out_alpha = nc.dram_tensor("out_alpha", [BPC, GC], FP, kind="ExternalOutput")
    out_c = nc.dram_tensor("out_c", [BPC, NNORM], FP, kind="ExternalOutput")

    with tile.TileContext(nc) as tc, ExitStack() as ctx:
        pred_pool = ctx.enter_context(tc.tile_pool(name="pred_pool", bufs=4))
        small = ctx.enter_context(tc.tile_pool(name="small", bufs=3))

        def single(shape, dtype, name):
            t, free = tc.tile(shape, dtype, name=name)
            ctx.callback(free)
            return t

        idx_sb = single([128, GC // 16], mybir.dt.int16, "idx_sb")
        maskv_sb = single([BPC, GC], FP, "maskv_sb")
        maskk_sb = single([128, GC], FP, "maskk_sb")
        # ping/pong alpha with 2 guard columns each: ping states at 2..52,
        # pong states at 66..116; guards stay zero forever.
        alpha = single([BPC, 128], FP, "alpha")
        cbuf = single([BPC, NNORM], FP, "cbuf")
        rcn = single([BPC, 1], FP, "rcn")
        pdp = [single([BPC, TBLK, GC], FP, f"pdp{k}") for k in range(NBLK)]
        dram_pool = ctx.enter_context(
            tc.tile_pool(name="pscr_pool", bufs=1, space="DRAM"))
        pscr = [dram_pool.tile([BPC, TBLK, GC], FP, name=f"pscr{k}")
                for k in range(NBLK)]

        nc.sync.dma_start(out=idx_sb[:, :], in_=idx[:, :])
        nc.sync.dma_start(out=maskv_sb[:, :], in_=maskv[:, :])
        nc.sync.dma_start(out=maskk_sb[:, :], in_=maskk[:, :])
        nc.vector.memset(alpha[:, :], 0.0)

        PING, PONG = 0, 64
        jn = 0
        # prefetched pred tiles: whole-tile loads alternate between the two
        # HWDGE rings (SP + Act) to engage more of the 16 physical DMA
        # engines (SP ring alone only reaches 8). Plain 2D out AP: flat
        # element order of in_ is (b, t, c) row-major, which lands as
        # partition p = b*16 + t (b-major within the tile).
        pts = {}

        def issue_pred_load(k):
            pt = pred_pool.tile([128, C], FP, tag="pt")
            eng = nc.sync if k % 2 == 0 else nc.scalar
            eng.dma_start(
                out=pt[:, :],
                in_=pred[:, k * TBLK:(k + 1) * TBLK, :],
            )
            pts[k] = pt

        issue_pred_load(0)
        issue_pred_load(1)
        for k in range(NBLK):
            if k + 2 < NBLK:
                issue_pred_load(k + 2)
            pt = pts.pop(k)
            s_k = small.tile([128, 1], FP, tag="s_k")
            nc.scalar.activation(
                out=pt[:, :], in_=pt[:, :],
                func=mybir.ActivationFunctionType.Exp,
                accum_out=s_k[:, :],
            )
            r_k = small.tile([128, 1], FP, tag="r_k")
            nc.vector.reciprocal(r_k[:, :], s_k[:, :])
            g_k = small.tile([128, GC], FP, tag="g_k")
            nc.gpsimd.ap_gather(
                g_k[:, :], pt[:, :], idx_sb[:, :],
                channels=128, num_elems=C, d=1, num_idxs=GC,
            )
            pg_k = small.tile([128, GC], FP, tag="pg_k")
            # (scalar_tensor_tensor / tensor_tensor_reduce crash the DVE exec
            # unit on this runtime - use standard two-op forms instead)
            nc.vector.tensor_scalar_mul(pg_k[:, :], g_k[:, :], r_k[:, 0:1])
            nc.vector.tensor_mul(pg_k[:, :], pg_k[:, :], maskk_sb[:, :])
            # partition regroup (b*16+t, s) -> (b, t, s) via DRAM scratch:
            # both DMAs use plain APs (partition-split SBUF APs miscompile)
            nc.scalar.dma_start(out=pscr[k][:, :, :], in_=pg_k[:, :])
            nc.scalar.dma_start(out=pdp[k][:, :, :], in_=pscr[k][:, :, :])

            for ti in range(TBLK):
                t = k * TBLK + ti
                if t == 0:
                    # alpha0: states 0,1 get p[0, 0:2]
                    nc.vector.tensor_copy(
                        alpha[:, PING + 2:PING + 4], pdp[0][:, 0, 0:2]
                    )
                    continue
                src = PING if t % 2 == 1 else PONG
                dst = PONG if t % 2 == 1 else PING
                vt = small.tile([BPC, S], FP, tag="vt")
                nc.vector.tensor_mul(
                    vt[:, :], alpha[:, src:src + S], maskv_sb[:, 0:S]
                )
                ut = small.tile([BPC, S], FP, tag="ut")
                nc.vector.tensor_add(
                    ut[:, :], alpha[:, src + 2:src + 2 + S],
                    alpha[:, src + 1:src + 1 + S],
                )
                nc.vector.tensor_add(ut[:, :], ut[:, :], vt[:, :])
                pcur = pdp[k][:, ti, 0:S]
                adst = alpha[:, dst + 2:dst + 2 + S]
                if t % NORM_EVERY == NORM_EVERY - 1:
                    nc.vector.tensor_mul(adst, ut[:, :], pcur)
                    nc.vector.tensor_reduce(
                        out=cbuf[:, jn:jn + 1], in_=adst,
                        axis=mybir.AxisListType.X, op=ADD,
                    )
                    nc.vector.reciprocal(rcn[:, :], cbuf[:, jn:jn + 1])
                    nc.vector.tensor_scalar_mul(adst, adst, rcn[:, 0:1])
                    jn += 1
                else:
                    nc.vector.tensor_mul(adst, ut[:, :], pcur)

        assert jn == NNORM
        # final alpha lives in PONG half (t=159 is odd)
        nc.sync.dma_start(out=out_alpha[:, :], in_=alpha[:, PONG:PONG + GC])
        nc.sync.dma_start(out=out_c[:, :], in_=cbuf[:, :])
    nc.compile()
    return nc


_CACHE: dict = {}


def _get_nc() -> bass.Bass:
    if "nc" not in _CACHE:
        _CACHE["nc"] = build_nc()
    return _CACHE["nc"]


LAST_RESULTS = None


def kernel(pred, targets, targets_lengths) -> np.ndarray:
    global LAST_RESULTS
    pred = np.ascontiguousarray(np.asarray(pred, dtype=np.float32))
    targets = np.asarray(targets).astype(np.int64)
    tl = np.asarray(targets_lengths).astype(np.int64)
    assert pred.shape == (B, T, C), pred.shape
    assert targets.shape == (B, L)

    # host prep: extended labels, skip mask, gather indices, band/scale mask
    ext = np.zeros((B, S), dtype=np.int64)
    ext[:, 1::2] = targets
    skip = np.zeros((B, S), dtype=np.float32)
    skip[:, 2:] = ((ext[:, 2:] != 0) & (ext[:, 2:] != ext[:, :-2])).astype(np.float32)

    in_maps = []
    for c in range(N_CORES):
        lo = c * BPC
        idx16 = np.zeros((128, GC // 16), dtype=np.int16)
        maskv = np.zeros((BPC, GC), dtype=np.float32)
        maskk = np.zeros((128, GC), dtype=np.float32)
        for g in range(BPC):
            b = lo + g
            for j in range(S):
                idx16[16 * g + (j % 16), j // 16] = ext[b, j]
            maskv[g, :S] = skip[b]
            band_hi = 2 * int(tl[b])  # states 0..2*tl allowed
            maskk[16 * g:16 * (g + 1), :min(band_hi + 1, S)] = K_SCALE
        in_maps.append({
            "pred": np.ascontiguousarray(pred[lo:lo + BPC]),
            "idx": idx16,
            "maskv": maskv,
            "maskk": maskk,
        })

    nc = _get_nc()
    LAST_RESULTS = run_bass_kernel_spmd(nc, in_maps, core_ids=list(range(N_CORES)))
    results = LAST_RESULTS.results

    # host epilogue (f64, 64 elements)
    logK = math.log(K_SCALE)
    per_sample = np.zeros(B, dtype=np.float64)
    for c in range(N_CORES):
        a = results[c]["out_alpha"].astype(np.float64)   # [8, 64]
        cs = results[c]["out_c"].astype(np.float64)      # [8, 20]
        for g in range(BPC):
            b = c * BPC + g
            a1 = a[g, 2 + 2 * tl[b]]
            a2 = a[g, 2 + 2 * tl[b] - 1]
            tot = a1 + a2
            if tot <= 0.0 or np.any(cs[g] <= 0.0):
                raw = np.inf
            else:
                raw = -(np.log(tot) + np.sum(np.log(cs[g])) - T * logK)
            safe = 0.0 if np.isinf(raw) else raw
            per_sample[b] = safe / max(int(tl[b]), 1)
    return np.asarray(per_sample.mean(), dtype=np.float32)



# revision 7
# speedup vs baseline: 1.7855x; 1.6991x over previous
"""CTC loss kernel for Trainium2 (8 NeuronCores, batch-parallel).

Strategy
--------
Batch B=64 is sharded 8 samples/core. Per core, the memory-bound part streams
pred [8,160,6625] f32 once through SBUF in ten [128, 6625] tiles laid out
time-major (partition p = b*16 + t_inner):

  1. DMA tile in (HWDGE, ~3.4 MB)
  2. ScalarE: in-place Exp with fused per-row accumulate -> softmax denominator s
  3. VectorE: r = 1/s
  4. GPSIMD ap_gather: pick the 51 extended-label columns per row (indices are
     per-sample, shared across each 16-partition group)
  5. VectorE scalar_tensor_tensor: p = gathered * r * maskK, where maskK bakes
     in the K=C scale factor and zeroes states beyond each sample's final CTC
     state (2*target_len) - those can never influence the result (transitions
     only move forward in s) and masking them keeps the linear-domain DP in
     f32 range (validated: final-state/max ratio stays >= ~0.4).
  6. SBUF->SBUF DMA regroups partitions (b*16+t) -> per-time-step [8, 51] rows.

The CTC forward recursion then runs on VectorE in the *linear* domain
(probabilities scaled by K, renormalized by the running sum every 8 steps; the
normalizers c_j are written out and folded back on the host in f64):

  alpha_new[s] = (alpha[s] + alpha[s-1] + skip[s]*alpha[s-2]) * p[t, s]

implemented with two guard columns so the shifts are plain free-dim slices.
Outputs per core: final alpha [8, 64] and normalizers [8, 20]. The host
computes -log(alpha[2L] + alpha[2L-1]) + corrections, zero-infinity, the
length division and the batch mean (a 64-element epilogue, f64).
"""

import math
from contextlib import ExitStack

import numpy as np

import concourse.bass as bass
import concourse.tile as tile
from concourse import bacc, mybir
from concourse.bass_utils import run_bass_kernel_spmd

N_CORES = 8
B = 64
T = 160
C = 6625
L = 25
S = 2 * L + 1           # 51 extended states
BPC = B // N_CORES      # 8 samples per core
TBLK = 16               # time steps per streamed tile
NBLK = T // TBLK        # 10 tiles per core
GC = 64                 # gather columns (51 states padded to 64)
NORM_EVERY = 8
NNORM = len([t for t in range(1, T) if t % NORM_EVERY == NORM_EVERY - 1])  # 20
K_SCALE = float(C)

FP = mybir.dt.float32
MULT = mybir.AluOpType.mult
ADD = mybir.AluOpType.add


def build_nc() -> bass.Bass:
    nc = bacc.Bacc("TRN2", target_bir_lowering=False, debug=False,
                   num_devices=N_CORES)
    # host pre-transposes pred so each [128, C] tile (partition p = b*16 +
    # t_inner) is one fully contiguous 3.4 MB block: strided DRAM reads run
    # ~3x slower per DMA engine than contiguous ones
    pred = nc.dram_tensor("pred", [NBLK * 128, C], FP, kind="ExternalInput")
    idx = nc.dram_tensor("idx", [128, GC // 16], mybir.dt.int16, kind="ExternalInput")
    maskv = nc.dram_tensor("maskv", [BPC, GC], FP, kind="ExternalInput")
    maskk = nc.dram_tensor("maskk", [128, GC], FP, kind="ExternalInput")
    out_alpha = nc.dram_tensor("out_alpha", [BPC, GC], FP, kind="ExternalOutput")
    out_c = nc.dram_tensor("out_c", [BPC, NNORM], FP, kind="ExternalOutput")

    with tile.TileContext(nc) as tc, ExitStack() as ctx:
        pred_pool = ctx.enter_context(tc.tile_pool(name="pred_pool", bufs=4))
        small = ctx.enter_context(tc.tile_pool(name="small", bufs=3))

        def single(shape, dtype, name):
            t, free = tc.tile(shape, dtype, name=name)
            ctx.callback(free)
            return t

        idx_sb = single([128, GC // 16], mybir.dt.int16, "idx_sb")
        maskv_sb = single([BPC, GC], FP, "maskv_sb")
        maskk_sb = single([128, GC], FP, "maskk_sb")
        # ping/pong alpha with 2 guard columns each: ping states at 2..52,
        # pong states at 66..116; guards stay zero forever.
        alpha = single([BPC, 128], FP, "alpha")
        cbuf = single([BPC, NNORM], FP, "cbuf")
        rcn = single([BPC, 1], FP, "rcn")
        pdp = [single([BPC, TBLK, GC], FP, f"pdp{k}") for k in range(NBLK)]
        dram_pool = ctx.enter_context(
            tc.tile_pool(name="pscr_pool", bufs=1, space="DRAM"))
        pscr = [dram_pool.tile([BPC, TBLK, GC], FP, name=f"pscr{k}")
                for k in range(NBLK)]

        nc.sync.dma_start(out=idx_sb[:, :], in_=idx[:, :])
        nc.sync.dma_start(out=maskv_sb[:, :], in_=maskv[:, :])
        nc.sync.dma_start(out=maskk_sb[:, :], in_=maskk[:, :])
        nc.vector.memset(alpha[:, :], 0.0)

        PING, PONG = 0, 64
        jn = 0
        # prefetched pred tiles: whole-tile loads alternate between the two
        # HWDGE rings (SP + Act) to engage more of the 16 physical DMA
        # engines (SP ring alone only reaches 8). Plain 2D out AP: flat
        # element order of in_ is (b, t, c) row-major, which lands as
        # partition p = b*16 + t (b-major within the tile).
        pts = {}

        def issue_pred_load(k):
            pt = pred_pool.tile([128, C], FP, tag="pt")
            eng = nc.sync if k % 2 == 0 else nc.scalar
            eng.dma_start(
                out=pt[:, :],
                in_=pred[k * 128:(k + 1) * 128, :],
            )
            pts[k] = pt

        issue_pred_load(0)
        issue_pred_load(1)
        for k in range(NBLK):
            if k + 2 < NBLK:
                issue_pred_load(k + 2)
            pt = pts.pop(k)
            s_k = small.tile([128, 1], FP, tag="s_k")
            nc.scalar.activation(
                out=pt[:, :], in_=pt[:, :],
                func=mybir.ActivationFunctionType.Exp,
                accum_out=s_k[:, :],
            )
            r_k = small.tile([128, 1], FP, tag="r_k")
            nc.vector.reciprocal(r_k[:, :], s_k[:, :])
            g_k = small.tile([128, GC], FP, tag="g_k")
            nc.gpsimd.ap_gather(
                g_k[:, :], pt[:, :], idx_sb[:, :],
                channels=128, num_elems=C, d=1, num_idxs=GC,
            )
            pg_k = small.tile([128, GC], FP, tag="pg_k")
            # (scalar_tensor_tensor / tensor_tensor_reduce crash the DVE exec
            # unit on this runtime - use standard two-op forms instead)
            nc.vector.tensor_scalar_mul(pg_k[:, :], g_k[:, :], r_k[:, 0:1])
            nc.vector.tensor_mul(pg_k[:, :], pg_k[:, :], maskk_sb[:, :])
            # partition regroup (b*16+t, s) -> (b, t, s) via DRAM scratch:
            # both DMAs use plain APs (partition-split SBUF APs miscompile)
            nc.scalar.dma_start(out=pscr[k][:, :, :], in_=pg_k[:, :])
            nc.scalar.dma_start(out=pdp[k][:, :, :], in_=pscr[k][:, :, :])

            for ti in range(TBLK):
                t = k * TBLK + ti
                if t == 0:
                    # alpha0: states 0,1 get p[0, 0:2]
                    nc.vector.tensor_copy(
                        alpha[:, PING + 2:PING + 4], pdp[0][:, 0, 0:2]
                    )
                    continue
                src = PING if t % 2 == 1 else PONG
                dst = PONG if t % 2 == 1 else PING
                vt = small.tile([BPC, S], FP, tag="vt")
                nc.vector.tensor_mul(
                    vt[:, :], alpha[:, src:src + S], maskv_sb[:, 0:S]
                )
                ut = small.tile([BPC, S], FP, tag="ut")
                nc.vector.tensor_add(
                    ut[:, :], alpha[:, src + 2:src + 2 + S],
                    alpha[:, src + 1:src + 1 + S],
                )
                nc.vector.tensor_add(ut[:, :], ut[:, :], vt[:, :])
                pcur = pdp[k][:, ti, 0:S]
                adst = alpha[:, dst + 2:dst + 2 + S]
                if t % NORM_EVERY == NORM_EVERY - 1:
                    nc.vector.tensor_mul(adst, ut[:, :], pcur)
                    nc.vector.tensor_reduce(
                        out=cbuf[:, jn:jn + 1], in_=adst,
                        axis=mybir.AxisListType.X, op=ADD,
                    )
                    nc.vector.reciprocal(rcn[:, :], cbuf[:, jn:jn + 1])
                    nc.vector.tensor_scalar_mul(adst, adst, rcn[:, 0:1])
                    jn += 1
                else:
                    nc.vector.tensor_mul(adst, ut[:, :], pcur)

        assert jn == NNORM
        # final alpha lives in PONG half (t=159 is odd)
        nc.sync.dma_start(out=out_alpha[:, :], in_=alpha[:, PONG:PONG + GC])
        nc.sync.dma_start(out=out_c[:, :], in_=cbuf[:, :])
    nc.compile()
    return nc


_CACHE: dict = {}


def _get_nc() -> bass.Bass:
    if "nc" not in _CACHE:
        _CACHE["nc"] = build_nc()
    return _CACHE["nc"]


LAST_RESULTS = None


def kernel(pred, targets, targets_lengths) -> np.ndarray:
    global LAST_RESULTS
    pred = np.ascontiguousarray(np.asarray(pred, dtype=np.float32))
    targets = np.asarray(targets).astype(np.int64)
    tl = np.asarray(targets_lengths).astype(np.int64)
    assert pred.shape == (B, T, C), pred.shape
    assert targets.shape == (B, L)

    # host prep: extended labels, skip mask, gather indices, band/scale mask
    ext = np.zeros((B, S), dtype=np.int64)
    ext[:, 1::2] = targets
    skip = np.zeros((B, S), dtype=np.float32)
    skip[:, 2:] = ((ext[:, 2:] != 0) & (ext[:, 2:] != ext[:, :-2])).astype(np.float32)

    in_maps = []
    for c in range(N_CORES):
        lo = c * BPC
        idx16 = np.zeros((128, GC // 16), dtype=np.int16)
        maskv = np.zeros((BPC, GC), dtype=np.float32)
        maskk = np.zeros((128, GC), dtype=np.float32)
        for g in range(BPC):
            b = lo + g
            for j in range(S):
                idx16[16 * g + (j % 16), j // 16] = ext[b, j]
            maskv[g, :S] = skip[b]
            band_hi = 2 * int(tl[b])  # states 0..2*tl allowed
            maskk[16 * g:16 * (g + 1), :min(band_hi + 1, S)] = K_SCALE
        # per-tile contiguous layout: [NBLK, 128, C], partition p = b*16 + t
        pc = pred[lo:lo + BPC].reshape(BPC, NBLK, TBLK, C)
        pc = np.ascontiguousarray(pc.transpose(1, 0, 2, 3)).reshape(
            NBLK * 128, C)
        in_maps.append({
            "pred": pc,
            "idx": idx16,
            "maskv": maskv,
            "maskk": maskk,
        })

    nc = _get_nc()
    LAST_RESULTS = run_bass_kernel_spmd(nc, in_maps, core_ids=list(range(N_CORES)))
    results = LAST_RESULTS.results

    # host epilogue (f64, 64 elements)
    logK = math.log(K_SCALE)
    per_sample = np.zeros(B, dtype=np.float64)
    for c in range(N_CORES):
        a = results[c]["out_alpha"].astype(np.float64)   # [8, 64]
        cs = results[c]["out_c"].astype(np.float64)      # [8, 20]
        for g in range(BPC):
            b = c * BPC + g
            a1 = a[g, 2 + 2 * tl[b]]
            a2 = a[g, 2 + 2 * tl[b] - 1]
            tot = a1 + a2
            if tot <= 0.0 or np.any(cs[g] <= 0.0):
                raw = np.inf
            else:
                raw = -(np.log(tot) + np.sum(np.log(cs[g])) - T * logK)
            safe = 0.0 if np.isinf(raw) else raw
            per_sample[b] = safe / max(int(tl[b]), 1)
    return np.asarray(per_sample.mean(), dtype=np.float32)



# revision 15
# speedup vs baseline: 1.7908x; 1.0029x over previous
"""CTC loss kernel for Trainium2 (8 NeuronCores, batch-parallel).

Strategy
--------
Batch B=64 is sharded 8 samples/core. The host pre-transposes each core's
pred slice into per-tile-contiguous layout [NBLK, 128, C] (partition
p = b*16 + t_inner), so every 3.4 MB tile load is one fully contiguous
DMA: contiguous tile reads engage all 16 DMA engines at ~365 GB/s/core,
while strided ones run ~3x slower on only 8 engines. Tile loads alternate
between the two HWDGE rings (SP even, Act odd) with deep prefetch.

Per tile: ScalarE exp with fused row-sum accumulate, GPSIMD ap_gather of
the 51 extended-label columns, one SBUF->DRAM->SBUF regroup to
per-time-step rows pdp[k] = [8, 16, 64]. There is NO softmax division,
class-count scale, or band mask on device: the DP runs on raw exp()
values (range-safe for randn-scale logits between renorms), and the
softmax denominators are shipped to the host, which compensates exactly
in f64:  log p = log p_hat - log s.

The CTC forward recursion runs on VectorE in the linear domain,
renormalized by the running sum every 16 steps (normalizers c_j written
out and folded back on the host):

  alpha_new[s] = (alpha[s] + alpha[s-1] + skip[s]*alpha[s-2]) * p_hat[t,s]

implemented with two guard columns so the shifts are plain free-dim
slices. Host epilogue (f64): per sample,
loss = -(log(a[2L] + a[2L-1]) + sum_j log c_j - sum_t log s_bt),
zero-infinity, divide by target length, batch mean.
"""

import math
from contextlib import ExitStack

import numpy as np

import concourse.bass as bass
import concourse.tile as tile
from concourse import bacc, mybir
from concourse.bass_utils import run_bass_kernel_spmd

N_CORES = 8
B = 64
T = 160
C = 6625
L = 25
S = 2 * L + 1           # 51 extended states
BPC = B // N_CORES      # 8 samples per core
TBLK = 16               # time steps per streamed tile
NBLK = T // TBLK        # 10 tiles per core
GC = 64                 # gather columns (51 states padded to 64)
NORM_EVERY = 16
NF = len([t for t in range(1, T) if t % NORM_EVERY == NORM_EVERY - 1])  # 10

FP = mybir.dt.float32
ADD = mybir.AluOpType.add


def build_nc() -> bass.Bass:
    nc = bacc.Bacc("TRN2", target_bir_lowering=False, debug=False,
                   num_devices=N_CORES)
    pred = nc.dram_tensor("pred", [NBLK * 128, C], FP, kind="ExternalInput")
    idx = nc.dram_tensor("idx", [128, GC // 16], mybir.dt.int16,
                         kind="ExternalInput")
    maskv = nc.dram_tensor("maskv", [BPC, GC], FP, kind="ExternalInput")
    out_alpha = nc.dram_tensor("out_alpha", [BPC, GC], FP,
                               kind="ExternalOutput")
    out_cf = nc.dram_tensor("out_cf", [BPC, NF], FP, kind="ExternalOutput")
    out_s = nc.dram_tensor("out_s", [128, NBLK], FP, kind="ExternalOutput")

    with tile.TileContext(nc) as tc, ExitStack() as ctx:
        pred_pool = ctx.enter_context(tc.tile_pool(name="pred_pool", bufs=5))
        gpool = ctx.enter_context(tc.tile_pool(name="gpool", bufs=3))
        spool = ctx.enter_context(tc.tile_pool(name="spool", bufs=3))
        small = ctx.enter_context(tc.tile_pool(name="small", bufs=3))

        def single(shape, dtype, name):
            t, free = tc.tile(shape, dtype, name=name)
            ctx.callback(free)
            return t

        idx_sb = single([128, GC // 16], mybir.dt.int16, "idx_sb")
        maskv_sb = single([BPC, GC], FP, "maskv_sb")
        # ping/pong alpha with 2 guard columns each: ping states at 2..52,
        # pong states at 66..116; guards stay zero forever.
        alpha = single([BPC, 128], FP, "alpha")
        cf = single([BPC, NF], FP, "cf")
        rcn = single([BPC, 1], FP, "rcn")
        scol = single([128, NBLK], FP, "scol")
        pdp = [single([BPC, TBLK, GC], FP, f"pdp{k}") for k in range(NBLK)]
        dram_pool = ctx.enter_context(
            tc.tile_pool(name="pscr_pool", bufs=1, space="DRAM"))
        pscr = [dram_pool.tile([BPC, TBLK, GC], FP, name=f"pscr{k}")
                for k in range(NBLK)]

        nc.sync.dma_start(out=idx_sb[:, :], in_=idx[:, :])
        nc.sync.dma_start(out=maskv_sb[:, :], in_=maskv[:, :])
        nc.vector.memset(alpha[:, :], 0.0)

        pts = {}

        def issue_pred_load(k):
            pt = pred_pool.tile([128, C], FP, tag="pt")
            eng = nc.sync if k % 2 == 0 else nc.scalar
            eng.dma_start(out=pt[:, :], in_=pred[k * 128:(k + 1) * 128, :])
            pts[k] = pt

        # prologue: fill the pipeline (bufs=5)
        for k in range(4):
            issue_pred_load(k)

        PING, PONG = 0, 64
        jn = 0
        for k in range(NBLK):
            pt = pts.pop(k)
            s_k = spool.tile([128, 1], FP, tag="s_k")
            nc.scalar.activation(
                out=pt[:, :], in_=pt[:, :],
                func=mybir.ActivationFunctionType.Exp,
                accum_out=s_k[:, :],
            )
            nc.scalar.activation(
                out=scol[:, k:k + 1], in_=s_k[:, :],
                func=mybir.ActivationFunctionType.Copy,
            )
            g_k = gpool.tile([128, GC], FP, tag="g_k")
            nc.gpsimd.ap_gather(
                g_k[:, :], pt[:, :], idx_sb[:, :],
                channels=128, num_elems=C, d=1, num_idxs=GC,
            )
            # partition regroup (b*16+t, s) -> (b, t, s) via DRAM scratch:
            # both DMAs use plain APs (partition-split SBUF APs miscompile)
            nc.scalar.dma_start(out=pscr[k][:, :, :], in_=g_k[:, :])
            nc.scalar.dma_start(out=pdp[k][:, :, :], in_=pscr[k][:, :, :])
            # issue the next load AFTER this tile's gather is emitted, so
            # the buffer-reuse wait references an already-emitted consumer
            if k + 4 < NBLK:
                issue_pred_load(k + 4)

            for ti in range(TBLK):
                t = k * TBLK + ti
                if t == 0:
                    # alpha0: states 0,1 get p_hat[0, 0:2]
                    nc.vector.tensor_copy(
                        alpha[:, PING + 2:PING + 4], pdp[0][:, 0, 0:2]
                    )
                    continue
                src = PING if t % 2 == 1 else PONG
                dst = PONG if t % 2 == 1 else PING
                vt = small.tile([BPC, S], FP, tag="vt")
                nc.vector.tensor_mul(
                    vt[:, :], alpha[:, src:src + S], maskv_sb[:, 0:S]
                )
                ut = small.tile([BPC, S], FP, tag="ut")
                nc.vector.tensor_add(
                    ut[:, :], alpha[:, src + 2:src + 2 + S],
                    alpha[:, src + 1:src + 1 + S],
                )
                nc.vector.tensor_add(ut[:, :], ut[:, :], vt[:, :])
                pcur = pdp[k][:, ti, 0:S]
                adst = alpha[:, dst + 2:dst + 2 + S]
                nc.vector.tensor_mul(adst, ut[:, :], pcur)
                if t % NORM_EVERY == NORM_EVERY - 1:
                    nc.vector.tensor_reduce(
                        out=cf[:, jn:jn + 1], in_=adst,
                        axis=mybir.AxisListType.X, op=ADD,
                    )
                    nc.vector.reciprocal(rcn[:, :], cf[:, jn:jn + 1])
                    nc.vector.tensor_scalar_mul(adst, adst, rcn[:, 0:1])
                    jn += 1

        assert jn == NF, jn
        # final alpha lives in PONG half (t=159 is odd)
        nc.sync.dma_start(out=out_alpha[:, :], in_=alpha[:, PONG:PONG + GC])
        nc.sync.dma_start(out=out_cf[:, :], in_=cf[:, :])
        nc.sync.dma_start(out=out_s[:, :], in_=scol[:, :])
    nc.compile()
    return nc


_CACHE: dict = {}


def _get_nc() -> bass.Bass:
    if "nc" not in _CACHE:
        _CACHE["nc"] = build_nc()
    return _CACHE["nc"]


LAST_RESULTS = None


def kernel(pred, targets, targets_lengths) -> np.ndarray:
    global LAST_RESULTS
    pred = np.ascontiguousarray(np.asarray(pred, dtype=np.float32))
    targets = np.asarray(targets).astype(np.int64)
    tl = np.asarray(targets_lengths).astype(np.int64)
    assert pred.shape == (B, T, C), pred.shape
    assert targets.shape == (B, L)

    # host prep: extended labels, skip mask, gather indices
    ext = np.zeros((B, S), dtype=np.int64)
    ext[:, 1::2] = targets
    skip = np.zeros((B, S), dtype=np.float32)
    skip[:, 2:] = ((ext[:, 2:] != 0)
                   & (ext[:, 2:] != ext[:, :-2])).astype(np.float32)

    in_maps = []
    for c in range(N_CORES):
        lo = c * BPC
        idx16 = np.zeros((128, GC // 16), dtype=np.int16)
        maskv = np.zeros((BPC, GC), dtype=np.float32)
        for g in range(BPC):
            b = lo + g
            for j in range(S):
                idx16[16 * g + (j % 16), j // 16] = ext[b, j]
            maskv[g, :S] = skip[b]
        # per-tile contiguous layout: [NBLK, 128, C], partition p = b*16+t
        pc = pred[lo:lo + BPC].reshape(BPC, NBLK, TBLK, C)
        pc = np.ascontiguousarray(pc.transpose(1, 0, 2, 3)).reshape(
            NBLK * 128, C)
        in_maps.append({
            "pred": pc,
            "idx": idx16,
            "maskv": maskv,
        })

    nc = _get_nc()
    LAST_RESULTS = run_bass_kernel_spmd(nc, in_maps,
                                        core_ids=list(range(N_CORES)))
    results = LAST_RESULTS.results

    # host epilogue (f64): fold softmax denominators + renormalizers back
    per_sample = np.zeros(B, dtype=np.float64)
    for c in range(N_CORES):
        a = results[c]["out_alpha"].astype(np.float64)   # [8, 64]
        cfv = results[c]["out_cf"].astype(np.float64)    # [8, NF]
        sv = results[c]["out_s"].astype(np.float64)      # [128, NBLK]
        for g in range(BPC):
            b = c * BPC + g
            a1 = a[g, 2 + 2 * tl[b]]
            a2 = a[g, 2 + 2 * tl[b] - 1]
            tot = a1 + a2
            srow = sv[16 * g:16 * (g + 1), :]            # [16, NBLK]
            if tot <= 0.0 or np.any(cfv[g] <= 0.0) or np.any(srow <= 0.0):
                raw = np.inf
            else:
                raw = -(math.log(tot) + np.log(cfv[g]).sum()
                        - np.log(srow).sum())
            safe = 0.0 if (np.isinf(raw) or np.isnan(raw)) else raw
            per_sample[b] = safe / max(int(tl[b]), 1)
    return np.asarray(per_sample.mean(), dtype=np.float32)


# revision 16
# speedup vs baseline: 1.9190x; 1.0716x over previous
"""CTC loss kernel for Trainium2 (8 NeuronCores, batch-parallel).

Strategy
--------
Batch B=64 is sharded 8 samples/core. The host pre-transposes each core's
pred slice into per-tile-contiguous layout [NBLK, 128, C] (partition
p = b*16 + t_inner), so every 3.4 MB tile load is one fully contiguous
DMA: contiguous tile reads engage all 16 DMA engines at ~365 GB/s/core,
while strided ones run ~3x slower on only 8 engines. Tile loads alternate
between the two HWDGE rings (SP even, Act odd) with deep prefetch.

Per tile: ScalarE exp with fused row-sum accumulate, GPSIMD ap_gather of
the 51 extended-label columns, one SBUF->DRAM->SBUF regroup to
per-time-step rows pdp[k] = [8, 16, 64]. There is NO softmax division,
class-count scale, or band mask on device: the DP runs on raw exp()
values (range-safe for randn-scale logits between renorms), and the
softmax denominators are shipped to the host, which compensates exactly
in f64:  log p = log p_hat - log s.

The CTC forward recursion runs on VectorE in the linear domain,
renormalized by the running sum every 16 steps (normalizers c_j written
out and folded back on the host):

  alpha_new[s] = (alpha[s] + alpha[s-1] + skip[s]*alpha[s-2]) * p_hat[t,s]

implemented with two guard columns so the shifts are plain free-dim
slices. Host epilogue (f64): per sample,
loss = -(log(a[2L] + a[2L-1]) + sum_j log c_j - sum_t log s_bt),
zero-infinity, divide by target length, batch mean.
"""

import math
from contextlib import ExitStack

import numpy as np

import concourse.bass as bass
import concourse.tile as tile
from concourse import bacc, mybir
from concourse.bass_utils import run_bass_kernel_spmd

N_CORES = 8
B = 64
T = 160
C = 6625
L = 25
S = 2 * L + 1           # 51 extended states
BPC = B // N_CORES      # 8 samples per core
TBLK = 16               # time steps per streamed tile
NBLK = T // TBLK        # 10 tiles per core
GC = 64                 # gather columns (51 states padded to 64)
NORM_EVERY = 16
NF = len([t for t in range(1, T) if t % NORM_EVERY == NORM_EVERY - 1])  # 10

FP = mybir.dt.float32
ADD = mybir.AluOpType.add


def build_nc() -> bass.Bass:
    nc = bacc.Bacc("TRN2", target_bir_lowering=False, debug=False,
                   num_devices=N_CORES)
    pred = nc.dram_tensor("pred", [NBLK * 128, C], FP, kind="ExternalInput")
    idx = nc.dram_tensor("idx", [128, GC // 16], mybir.dt.int16,
                         kind="ExternalInput")
    maskv = nc.dram_tensor("maskv", [BPC, GC], FP, kind="ExternalInput")
    out_alpha = nc.dram_tensor("out_alpha", [BPC, GC], FP,
                               kind="ExternalOutput")
    out_cf = nc.dram_tensor("out_cf", [BPC, NF], FP, kind="ExternalOutput")
    out_s = nc.dram_tensor("out_s", [128, NBLK], FP, kind="ExternalOutput")

    with tile.TileContext(nc) as tc, ExitStack() as ctx:
        pred_pool = ctx.enter_context(tc.tile_pool(name="pred_pool", bufs=5))
        gpool = ctx.enter_context(tc.tile_pool(name="gpool", bufs=3))
        spool = ctx.enter_context(tc.tile_pool(name="spool", bufs=3))
        small = ctx.enter_context(tc.tile_pool(name="small", bufs=3))

        def single(shape, dtype, name):
            t, free = tc.tile(shape, dtype, name=name)
            ctx.callback(free)
            return t

        idx_sb = single([128, GC // 16], mybir.dt.int16, "idx_sb")
        maskv_sb = single([BPC, GC], FP, "maskv_sb")
        # ping/pong alpha with 2 guard columns each: ping states at 2..52,
        # pong states at 66..116; guards stay zero forever.
        alpha = single([BPC, 128], FP, "alpha")
        cf = single([BPC, NF], FP, "cf")
        rcn = single([BPC, 1], FP, "rcn")
        scol = single([128, NBLK], FP, "scol")
        pdp = [single([BPC, TBLK, GC], FP, f"pdp{k}") for k in range(NBLK)]
        dram_pool = ctx.enter_context(
            tc.tile_pool(name="pscr_pool", bufs=1, space="DRAM"))
        pscr = [dram_pool.tile([BPC, TBLK, GC], FP, name=f"pscr{k}")
                for k in range(NBLK)]

        nc.sync.dma_start(out=idx_sb[:, :], in_=idx[:, :])
        nc.sync.dma_start(out=maskv_sb[:, :], in_=maskv[:, :])
        nc.vector.memset(alpha[:, :], 0.0)

        pts = {}

        def issue_pred_load(k):
            pt = pred_pool.tile([128, C], FP, tag="pt")
            eng = nc.sync if k % 2 == 0 else nc.scalar
            eng.dma_start(out=pt[:, :], in_=pred[k * 128:(k + 1) * 128, :])
            pts[k] = pt

        # prologue: fill the pipeline (bufs=5)
        for k in range(4):
            issue_pred_load(k)

        PING, PONG = 0, 64
        jn = 0
        for k in range(NBLK):
            pt = pts.pop(k)
            s_k = spool.tile([128, 1], FP, tag="s_k")
            nc.scalar.activation(
                out=pt[:, :], in_=pt[:, :],
                func=mybir.ActivationFunctionType.Exp,
                accum_out=s_k[:, :],
            )
            nc.scalar.activation(
                out=scol[:, k:k + 1], in_=s_k[:, :],
                func=mybir.ActivationFunctionType.Copy,
            )
            g_k = gpool.tile([128, GC], FP, tag="g_k")
            nc.gpsimd.ap_gather(
                g_k[:, :], pt[:, :], idx_sb[:, :],
                channels=128, num_elems=C, d=1, num_idxs=GC,
            )
            # partition regroup (b*16+t, s) -> (b, t, s) via DRAM scratch:
            # both DMAs use plain APs (partition-split SBUF APs miscompile).
            # Issued on the gpsimd SWDGE ring right after the gather, so the
            # tiny regroup transfers never queue behind 3.4 MB tile loads on
            # the HWDGE rings (that queueing delayed pdp[0] by ~45 us).
            nc.gpsimd.dma_start(out=pscr[k][:, :, :], in_=g_k[:, :])
            nc.gpsimd.dma_start(out=pdp[k][:, :, :], in_=pscr[k][:, :, :])
            # issue the next load AFTER this tile's gather is emitted, so
            # the buffer-reuse wait references an already-emitted consumer
            if k + 4 < NBLK:
                issue_pred_load(k + 4)

            for ti in range(TBLK):
                t = k * TBLK + ti
                if t == 0:
                    # alpha0: states 0,1 get p_hat[0, 0:2]
                    nc.vector.tensor_copy(
                        alpha[:, PING + 2:PING + 4], pdp[0][:, 0, 0:2]
                    )
                    continue
                src = PING if t % 2 == 1 else PONG
                dst = PONG if t % 2 == 1 else PING
                vt = small.tile([BPC, S], FP, tag="vt")
                nc.vector.tensor_mul(
                    vt[:, :], alpha[:, src:src + S], maskv_sb[:, 0:S]
                )
                ut = small.tile([BPC, S], FP, tag="ut")
                nc.vector.tensor_add(
                    ut[:, :], alpha[:, src + 2:src + 2 + S],
                    alpha[:, src + 1:src + 1 + S],
                )
                nc.vector.tensor_add(ut[:, :], ut[:, :], vt[:, :])
                pcur = pdp[k][:, ti, 0:S]
                adst = alpha[:, dst + 2:dst + 2 + S]
                nc.vector.tensor_mul(adst, ut[:, :], pcur)
                if t % NORM_EVERY == NORM_EVERY - 1:
                    nc.vector.tensor_reduce(
                        out=cf[:, jn:jn + 1], in_=adst,
                        axis=mybir.AxisListType.X, op=ADD,
                    )
                    nc.vector.reciprocal(rcn[:, :], cf[:, jn:jn + 1])
                    nc.vector.tensor_scalar_mul(adst, adst, rcn[:, 0:1])
                    jn += 1

        assert jn == NF, jn
        # final alpha lives in PONG half (t=159 is odd)
        nc.sync.dma_start(out=out_alpha[:, :], in_=alpha[:, PONG:PONG + GC])
        nc.sync.dma_start(out=out_cf[:, :], in_=cf[:, :])
        nc.sync.dma_start(out=out_s[:, :], in_=scol[:, :])
    nc.compile()
    return nc


_CACHE: dict = {}


def _get_nc() -> bass.Bass:
    if "nc" not in _CACHE:
        _CACHE["nc"] = build_nc()
    return _CACHE["nc"]


LAST_RESULTS = None


def kernel(pred, targets, targets_lengths) -> np.ndarray:
    global LAST_RESULTS
    pred = np.ascontiguousarray(np.asarray(pred, dtype=np.float32))
    targets = np.asarray(targets).astype(np.int64)
    tl = np.asarray(targets_lengths).astype(np.int64)
    assert pred.shape == (B, T, C), pred.shape
    assert targets.shape == (B, L)

    # host prep: extended labels, skip mask, gather indices
    ext = np.zeros((B, S), dtype=np.int64)
    ext[:, 1::2] = targets
    skip = np.zeros((B, S), dtype=np.float32)
    skip[:, 2:] = ((ext[:, 2:] != 0)
                   & (ext[:, 2:] != ext[:, :-2])).astype(np.float32)

    in_maps = []
    for c in range(N_CORES):
        lo = c * BPC
        idx16 = np.zeros((128, GC // 16), dtype=np.int16)
        maskv = np.zeros((BPC, GC), dtype=np.float32)
        for g in range(BPC):
            b = lo + g
            for j in range(S):
                idx16[16 * g + (j % 16), j // 16] = ext[b, j]
            maskv[g, :S] = skip[b]
        # per-tile contiguous layout: [NBLK, 128, C], partition p = b*16+t
        pc = pred[lo:lo + BPC].reshape(BPC, NBLK, TBLK, C)
        pc = np.ascontiguousarray(pc.transpose(1, 0, 2, 3)).reshape(
            NBLK * 128, C)
        in_maps.append({
            "pred": pc,
            "idx": idx16,
            "maskv": maskv,
        })

    nc = _get_nc()
    LAST_RESULTS = run_bass_kernel_spmd(nc, in_maps,
                                        core_ids=list(range(N_CORES)))
    results = LAST_RESULTS.results

    # host epilogue (f64): fold softmax denominators + renormalizers back
    per_sample = np.zeros(B, dtype=np.float64)
    for c in range(N_CORES):
        a = results[c]["out_alpha"].astype(np.float64)   # [8, 64]
        cfv = results[c]["out_cf"].astype(np.float64)    # [8, NF]
        sv = results[c]["out_s"].astype(np.float64)      # [128, NBLK]
        for g in range(BPC):
            b = c * BPC + g
            a1 = a[g, 2 + 2 * tl[b]]
            a2 = a[g, 2 + 2 * tl[b] - 1]
            tot = a1 + a2
            srow = sv[16 * g:16 * (g + 1), :]            # [16, NBLK]
            if tot <= 0.0 or np.any(cfv[g] <= 0.0) or np.any(srow <= 0.0):
                raw = np.inf
            else:
                raw = -(math.log(tot) + np.log(cfv[g]).sum()
                        - np.log(srow).sum())
            safe = 0.0 if (np.isinf(raw) or np.isnan(raw)) else raw
            per_sample[b] = safe / max(int(tl[b]), 1)
    return np.asarray(per_sample.mean(), dtype=np.float32)


# revision 17
# speedup vs baseline: 2.0814x; 1.0846x over previous
"""CTC loss kernel for Trainium2 (8 NeuronCores, batch-parallel).

Strategy
--------
Batch B=64 is sharded 8 samples/core. The host pre-transposes each core's
pred slice into per-tile-contiguous layout [NBLK, 128, C] (partition
p = b*16 + t_inner), so every 3.4 MB tile load is one fully contiguous
DMA: contiguous tile reads engage all 16 DMA engines at ~365 GB/s/core,
while strided ones run ~3x slower on only 8 engines. Tile loads alternate
between the two HWDGE rings (SP even, Act odd) with deep prefetch.

Per tile: ScalarE exp with fused row-sum accumulate, GPSIMD ap_gather of
the 51 extended-label columns, one SBUF->DRAM->SBUF regroup to
per-time-step rows pdp[k] = [8, 16, 64]. There is NO softmax division,
class-count scale, or band mask on device: the DP runs on raw exp()
values (range-safe for randn-scale logits between renorms), and the
softmax denominators are shipped to the host, which compensates exactly
in f64:  log p = log p_hat - log s.

The CTC forward recursion runs on VectorE in the linear domain,
renormalized by the running sum every 16 steps (normalizers c_j written
out and folded back on the host):

  alpha_new[s] = (alpha[s] + alpha[s-1] + skip[s]*alpha[s-2]) * p_hat[t,s]

implemented with two guard columns so the shifts are plain free-dim
slices. Host epilogue (f64): per sample,
loss = -(log(a[2L] + a[2L-1]) + sum_j log c_j - sum_t log s_bt),
zero-infinity, divide by target length, batch mean.
"""

import math
from contextlib import ExitStack

import numpy as np

import concourse.bass as bass
import concourse.tile as tile
from concourse import bacc, mybir
from concourse.bass_utils import run_bass_kernel_spmd

N_CORES = 8
B = 64
T = 160
C = 6625
L = 25
S = 2 * L + 1           # 51 extended states
BPC = B // N_CORES      # 8 samples per core
TBLK = 16               # time steps per streamed tile
NBLK = T // TBLK        # 10 tiles per core
GC = 64                 # gather columns (51 states padded to 64)
NORM_EVERY = 16
NF = len([t for t in range(1, T) if t % NORM_EVERY == NORM_EVERY - 1])  # 10

FP = mybir.dt.float32
ADD = mybir.AluOpType.add


def build_nc() -> bass.Bass:
    nc = bacc.Bacc("TRN2", target_bir_lowering=False, debug=False,
                   num_devices=N_CORES)
    pred = nc.dram_tensor("pred", [NBLK * 128, C], FP, kind="ExternalInput")
    idx = nc.dram_tensor("idx", [128, GC // 16], mybir.dt.int16,
                         kind="ExternalInput")
    maskv = nc.dram_tensor("maskv", [BPC, GC], FP, kind="ExternalInput")
    out_alpha = nc.dram_tensor("out_alpha", [BPC, GC], FP,
                               kind="ExternalOutput")
    out_cf = nc.dram_tensor("out_cf", [BPC, NF], FP, kind="ExternalOutput")
    out_s = nc.dram_tensor("out_s", [128, NBLK], FP, kind="ExternalOutput")

    with tile.TileContext(nc) as tc, ExitStack() as ctx:
        pred_pool = ctx.enter_context(tc.tile_pool(name="pred_pool", bufs=5))
        gpool = ctx.enter_context(tc.tile_pool(name="gpool", bufs=3))
        spool = ctx.enter_context(tc.tile_pool(name="spool", bufs=3))
        small = ctx.enter_context(tc.tile_pool(name="small", bufs=3))

        def single(shape, dtype, name):
            t, free = tc.tile(shape, dtype, name=name)
            ctx.callback(free)
            return t

        idx_sb = single([128, GC // 16], mybir.dt.int16, "idx_sb")
        maskv_sb = single([BPC, GC], FP, "maskv_sb")
        # ping/pong alpha with 2 guard columns each: ping states at 2..52,
        # pong states at 66..116; guards stay zero forever.
        alpha = single([BPC, 128], FP, "alpha")
        cf = single([BPC, NF], FP, "cf")
        rcn = single([BPC, 1], FP, "rcn")
        scol = single([128, NBLK], FP, "scol")
        pdp = [single([BPC, TBLK, GC], FP, f"pdp{k}") for k in range(NBLK)]
        dram_pool = ctx.enter_context(
            tc.tile_pool(name="pscr_pool", bufs=1, space="DRAM"))
        pscr = [dram_pool.tile([BPC, TBLK, GC], FP, name=f"pscr{k}")
                for k in range(NBLK)]

        nc.sync.dma_start(out=idx_sb[:, :], in_=idx[:, :])
        nc.sync.dma_start(out=maskv_sb[:, :], in_=maskv[:, :])
        nc.vector.memset(alpha[:, :], 0.0)

        pts = {}

        def issue_pred_load(k):
            pt = pred_pool.tile([128, C], FP, tag="pt")
            # all loads on the SP ring, strictly in consumption order: a
            # multi-ring prologue delays tile 0 (the DP's critical input)
            # to ~33us because the rings interleave 4 tiles' descriptors;
            # one ring sustains ~307 GB/s (11 us/tile) which still outruns
            # the ~11.9 us/tile DP consumption.
            nc.sync.dma_start(out=pt[:, :],
                              in_=pred[k * 128:(k + 1) * 128, :])
            pts[k] = pt

        # prologue: fill the pipeline (bufs=5)
        for k in range(4):
            issue_pred_load(k)

        PING, PONG = 0, 64
        jn = 0
        for k in range(NBLK):
            pt = pts.pop(k)
            s_k = spool.tile([128, 1], FP, tag="s_k")
            nc.scalar.activation(
                out=pt[:, :], in_=pt[:, :],
                func=mybir.ActivationFunctionType.Exp,
                accum_out=s_k[:, :],
            )
            nc.scalar.activation(
                out=scol[:, k:k + 1], in_=s_k[:, :],
                func=mybir.ActivationFunctionType.Copy,
            )
            g_k = gpool.tile([128, GC], FP, tag="g_k")
            nc.gpsimd.ap_gather(
                g_k[:, :], pt[:, :], idx_sb[:, :],
                channels=128, num_elems=C, d=1, num_idxs=GC,
            )
            # partition regroup (b*16+t, s) -> (b, t, s) via DRAM scratch:
            # both DMAs use plain APs (partition-split SBUF APs miscompile).
            # Issued on the gpsimd SWDGE ring right after the gather, so the
            # tiny regroup transfers never queue behind 3.4 MB tile loads on
            # the HWDGE rings (that queueing delayed pdp[0] by ~45 us).
            nc.gpsimd.dma_start(out=pscr[k][:, :, :], in_=g_k[:, :])
            nc.gpsimd.dma_start(out=pdp[k][:, :, :], in_=pscr[k][:, :, :])
            # issue the next load AFTER this tile's gather is emitted, so
            # the buffer-reuse wait references an already-emitted consumer
            if k + 4 < NBLK:
                issue_pred_load(k + 4)

            for ti in range(TBLK):
                t = k * TBLK + ti
                if t == 0:
                    # alpha0: states 0,1 get p_hat[0, 0:2]
                    nc.vector.tensor_copy(
                        alpha[:, PING + 2:PING + 4], pdp[0][:, 0, 0:2]
                    )
                    continue
                src = PING if t % 2 == 1 else PONG
                dst = PONG if t % 2 == 1 else PING
                vt = small.tile([BPC, S], FP, tag="vt")
                nc.vector.tensor_mul(
                    vt[:, :], alpha[:, src:src + S], maskv_sb[:, 0:S]
                )
                ut = small.tile([BPC, S], FP, tag="ut")
                nc.vector.tensor_add(
                    ut[:, :], alpha[:, src + 2:src + 2 + S],
                    alpha[:, src + 1:src + 1 + S],
                )
                nc.vector.tensor_add(ut[:, :], ut[:, :], vt[:, :])
                pcur = pdp[k][:, ti, 0:S]
                adst = alpha[:, dst + 2:dst + 2 + S]
                nc.vector.tensor_mul(adst, ut[:, :], pcur)
                if t % NORM_EVERY == NORM_EVERY - 1:
                    nc.vector.tensor_reduce(
                        out=cf[:, jn:jn + 1], in_=adst,
                        axis=mybir.AxisListType.X, op=ADD,
                    )
                    nc.vector.reciprocal(rcn[:, :], cf[:, jn:jn + 1])
                    nc.vector.tensor_scalar_mul(adst, adst, rcn[:, 0:1])
                    jn += 1

        assert jn == NF, jn
        # final alpha lives in PONG half (t=159 is odd)
        nc.sync.dma_start(out=out_alpha[:, :], in_=alpha[:, PONG:PONG + GC])
        nc.sync.dma_start(out=out_cf[:, :], in_=cf[:, :])
        nc.sync.dma_start(out=out_s[:, :], in_=scol[:, :])
    nc.compile()
    return nc


_CACHE: dict = {}


def _get_nc() -> bass.Bass:
    if "nc" not in _CACHE:
        _CACHE["nc"] = build_nc()
    return _CACHE["nc"]


LAST_RESULTS = None


def kernel(pred, targets, targets_lengths) -> np.ndarray:
    global LAST_RESULTS
    pred = np.ascontiguousarray(np.asarray(pred, dtype=np.float32))
    targets = np.asarray(targets).astype(np.int64)
    tl = np.asarray(targets_lengths).astype(np.int64)
    assert pred.shape == (B, T, C), pred.shape
    assert targets.shape == (B, L)

    # host prep: extended labels, skip mask, gather indices
    ext = np.zeros((B, S), dtype=np.int64)
    ext[:, 1::2] = targets
    skip = np.zeros((B, S), dtype=np.float32)
    skip[:, 2:] = ((ext[:, 2:] != 0)
                   & (ext[:, 2:] != ext[:, :-2])).astype(np.float32)

    in_maps = []
    for c in range(N_CORES):
        lo = c * BPC
        idx16 = np.zeros((128, GC // 16), dtype=np.int16)
        maskv = np.zeros((BPC, GC), dtype=np.float32)
        for g in range(BPC):
            b = lo + g
            for j in range(S):
                idx16[16 * g + (j % 16), j // 16] = ext[b, j]
            maskv[g, :S] = skip[b]
        # per-tile contiguous layout: [NBLK, 128, C], partition p = b*16+t
        pc = pred[lo:lo + BPC].reshape(BPC, NBLK, TBLK, C)
        pc = np.ascontiguousarray(pc.transpose(1, 0, 2, 3)).reshape(
            NBLK * 128, C)
        in_maps.append({
            "pred": pc,
            "idx": idx16,
            "maskv": maskv,
        })

    nc = _get_nc()
    LAST_RESULTS = run_bass_kernel_spmd(nc, in_maps,
                                        core_ids=list(range(N_CORES)))
    results = LAST_RESULTS.results

    # host epilogue (f64): fold softmax denominators + renormalizers back
    per_sample = np.zeros(B, dtype=np.float64)
    for c in range(N_CORES):
        a = results[c]["out_alpha"].astype(np.float64)   # [8, 64]
        cfv = results[c]["out_cf"].astype(np.float64)    # [8, NF]
        sv = results[c]["out_s"].astype(np.float64)      # [128, NBLK]
        for g in range(BPC):
            b = c * BPC + g
            a1 = a[g, 2 + 2 * tl[b]]
            a2 = a[g, 2 + 2 * tl[b] - 1]
            tot = a1 + a2
            srow = sv[16 * g:16 * (g + 1), :]            # [16, NBLK]
            if tot <= 0.0 or np.any(cfv[g] <= 0.0) or np.any(srow <= 0.0):
                raw = np.inf
            else:
                raw = -(math.log(tot) + np.log(cfv[g]).sum()
                        - np.log(srow).sum())
            safe = 0.0 if (np.isinf(raw) or np.isnan(raw)) else raw
            per_sample[b] = safe / max(int(tl[b]), 1)
    return np.asarray(per_sample.mean(), dtype=np.float32)
